# revision 34
# baseline (speedup 1.0000x reference)
"""Causal GQA self-attention (B=2, C=2048, D=2048, 16 heads / 4 KV groups)
as a Bass/Tile SPMD kernel on 8 trn2 NeuronCores.

Sharding: core i handles batch b = i // 4, KV group g = i % 4 (4 query
heads + 1 KV head). Each core computes a partial output
x[b] @ Wq_g -> attention -> attn @ Wo_g_rows; the host sums the 4
partials per batch (the Wo-row split means partials add exactly).

Layout choices (matmuls run as float32r):
- host feeds x transposed (xT: D x C) so both projection operands are
  naturally k-major; projections emit head-dim-major qT/kT/vT tiles.
- Wq/Wk columns are permuted per head to [even dims | odd dims] so RoPE
  becomes full-width elementwise ops: rot = q*CC + swap(q)*SS with
  CC=[cos;cos], SS=[-sin;sin]; swap(q) made by two ACT partition-shift
  copies. The softmax scale 1/sqrt(D) is folded into Wq on the host.
- scores are computed transposed (keys on partitions, queries free) so
  probs feed the attnT matmul without transposes; softmax denominators
  come from a ones-column matmul; 1/sum via ACT exp(-ln(s)); the
  per-query normalizer is broadcast across partitions with a K=1
  outer-product matmul into the same PSUM bank the sums used.
- v alone is transposed (PE transpose via identity) to token-major.
- attention runs head pairs so consecutive matmuls share the stationary
  kT/v/ones operands; causal structure is exploited by computing only
  the valid query subrange of every scores/probs/attn/sums tile.
- DMA order is tuned so the first projection matmul starts ~13us in
  (wq and chunk-0 xT interleaved, then later chunks, then wk/wv; small
  constants ride the idle SWDGE queue).
- out-projection emission is deferred two query chunks behind attention
  so its exp-independent matmuls fill the PE stalls that the
  scores->exp->attn dependency chain would otherwise expose.
"""
import sys
import types

import numpy as np

import concourse.bass as bass
import concourse.mybir as mybir
import concourse.tile as tile
from concourse.bass_utils import run_bass_kernel_spmd

F32R = mybir.dt.float32r
F32 = mybir.dt.float32
AF = mybir.ActivationFunctionType
OP = mybir.AluOpType

C = 2048
D = 2048
HD = 128
HPC = 4          # query heads per core
CH = 512         # token chunk
NCH = C // CH    # 4
KB = D // 128    # 16 k blocks
TRACE = False    # set by test harness to collect NTFF exec time

# ---------------------------------------------------------------- shims ----


def _install_ntff_shim():
    """Provide antenv.axon_hooks (missing on this image) so trace=True works."""
    if "antenv.axon_hooks" in sys.modules:
        return
    try:
        import antenv
        from trn_agent_boot.trn_boot import _ntff_profile_via_ctypes
    except ImportError:
        return
    mod = types.ModuleType("antenv.axon_hooks")
    state = {"hook": None}
    mod.set_axon_ntff_profile_hook = lambda h: state.__setitem__("hook", h)
    mod.get_axon_ntff_profile_hook = lambda: state["hook"]
    sys.modules["antenv.axon_hooks"] = mod
    antenv.axon_hooks = mod
    mod.set_axon_ntff_profile_hook(
        _ntff_profile_via_ctypes("/opt/axon/libaxon_pjrt.so")
    )


_wsplit_counter = [0]


def _split_excess_waits(nc, limit: int = 1):
    """This walrus build allows few sync waits per instruction (fused f32r
    matmuls take one); move extras onto same-engine NoOp carriers."""
    for f in nc.m.functions:
        for bb in f.blocks:
            insts = list(bb.instructions)
            if not any(
                i.sync_info is not None and len(i.sync_info.on_wait) > limit
                for i in insts
            ):
                continue
            new_insts = []
            for inst in insts:
                si = inst.sync_info
                if si is not None and len(si.on_wait) > limit:
                    waits = list(si.on_wait)
                    keep, extra = waits[:limit], waits[limit:]
                    for w in extra:
                        _wsplit_counter[0] += 1
                        nop = mybir.InstNoOp(
                            name=f"I-wsplit-{_wsplit_counter[0]}",
                            ins=[],
                            outs=[],
                            engine=inst.engine,
                        )
                        nop.sync_info = mybir.SyncInfo(on_wait=[w], on_update=[])
                        new_insts.append(nop)
                    inst.sync_info = mybir.SyncInfo(
                        on_wait=keep, on_update=list(si.on_update)
                    )
                new_insts.append(inst)
            bb.instructions = new_insts


# ------------------------------------------------------------- program ----


def _build():
    nc = bass.Bass(target_bir_lowering=False)

    xT = nc.dram_tensor("xT", [D, C], F32R, kind="ExternalInput")
    wq = nc.dram_tensor("wq", [D, HPC * HD], F32R, kind="ExternalInput")
    wk = nc.dram_tensor("wk", [D, HD], F32R, kind="ExternalInput")
    wv = nc.dram_tensor("wv", [D, HD], F32R, kind="ExternalInput")
    wo = nc.dram_tensor("wo", [HPC * HD, D], F32R, kind="ExternalInput")
    cc = nc.dram_tensor("cc", [128, C], F32, kind="ExternalInput")
    ss = nc.dram_tensor("ss", [128, C], F32, kind="ExternalInput")
    eye = nc.dram_tensor("eye", [128, 128], F32R, kind="ExternalInput")
    trim = nc.dram_tensor("trim", [128, 128], F32R, kind="ExternalInput")
    onesc = nc.dram_tensor("onesc", [128, 1], F32R, kind="ExternalInput")
    onesr = nc.dram_tensor("onesr", [1, 128], F32R, kind="ExternalInput")
    out = nc.dram_tensor("out", [C, D], F32, kind="ExternalOutput")

    wq_r = wq.rearrange("(kb p) n -> p kb n", p=128)
    wk_r = wk.rearrange("(kb p) n -> p kb n", p=128)
    wv_r = wv.rearrange("(kb p) n -> p kb n", p=128)
    wo_r = wo.rearrange("(hb p) e -> p hb e", p=128)
    xT_r = xT.rearrange("(kb p) m -> p kb m", p=128)

    with tile.TileContext(nc) as tc:
        with (
            tc.tile_pool(name="const", bufs=1) as cpool,
            tc.tile_pool(name="acts", bufs=1) as apool,
        ):
            # ---- resident constants / activations (data DMAs emitted in
            # phase P interleaved with xT so the first matmul starts early;
            # low-priority constants go on the gpsimd SWDGE queue)
            wk_sb = cpool.tile([128, KB, HD], F32R)
            wv_sb = cpool.tile([128, KB, HD], F32R)
            cc_sb = cpool.tile([128, C], F32)
            ss_sb = cpool.tile([128, C], F32)
            eye_sb = cpool.tile([128, 128], F32R)
            nc.gpsimd.dma_start(eye_sb[:], eye[:])
            trim_sb = cpool.tile([128, 128], F32R)
            nc.gpsimd.dma_start(trim_sb[:], trim[:])
            onesc_sb = cpool.tile([128, 1], F32R)
            nc.gpsimd.dma_start(onesc_sb[:], onesc[:])
            onesr_sb = cpool.tile([1, 128], F32R)
            nc.gpsimd.dma_start(onesr_sb[:], onesr[:])

            qT_sb = apool.tile([128, HPC, C], F32R)    # head-dim major q
            kT_sb = apool.tile([128, C], F32R)         # head-dim major k
            v_sb = apool.tile([128, KB, HD], F32R)     # token-major v

            # ---- phase P: projections + RoPE, two chunks per weight load
            with (
                tc.tile_pool(name="wqp", bufs=1) as wqpool,
                tc.tile_pool(name="xt", bufs=4) as xtpool,
                tc.tile_pool(name="rope", bufs=2) as rpool,
                tc.tile_pool(name="psproj", bufs=1, space="PSUM") as pspj,
                tc.tile_pool(name="pstr", bufs=1, space="PSUM") as pstr,
            ):
                wq_sb = wqpool.tile([128, KB, HPC * HD], F32R)

                def load_xt(t):
                    halves_x = []
                    for xh in range(2):
                        xt_t = xtpool.tile(
                            [128, KB // 2, CH], F32R, tag="xt", name="xt"
                        )
                        for kb2 in range(KB // 2):
                            kb = xh * (KB // 2) + kb2
                            nc.sync.dma_start(
                                xt_t[:, kb2, :],
                                xT_r[:, kb, t * CH:(t + 1) * CH],
                            )
                        halves_x.append(xt_t)
                    return halves_x

                # DMA order tuned for an early first matmul: wq and chunk-0
                # xT interleaved, then chunk-1 xT, then wk/wv (used only by
                # the second half-pass), cc/ss on the idle SWDGE queue.
                xt0a = xtpool.tile([128, KB // 2, CH], F32R, tag="xt", name="xt")
                xt0b = xtpool.tile([128, KB // 2, CH], F32R, tag="xt", name="xt")
                xt0 = [xt0a, xt0b]
                for kb in range(KB):
                    nc.sync.dma_start(wq_sb[:, kb, :], wq_r[:, kb, :])
                    nc.sync.dma_start(
                        xt0[kb // (KB // 2)][:, kb % (KB // 2), :],
                        xT_r[:, kb, 0:CH],
                    )
                nc.gpsimd.dma_start(cc_sb[:], cc[:])
                nc.gpsimd.dma_start(ss_sb[:], ss[:])
                prefetched = {0: xt0, 1: load_xt(1)}
                for kb in range(KB):
                    nc.sync.dma_start(wk_sb[:, kb, :], wk_r[:, kb, :])
                    nc.sync.dma_start(wv_sb[:, kb, :], wv_r[:, kb, :])

                def rope(ps, dst, ts):
                    qsw = rpool.tile([128, CH], F32, tag="qsw", name="qsw")
                    nc.scalar.copy(qsw[0:64, :], ps[64:128, :])
                    nc.scalar.copy(qsw[64:128, :], ps[0:64, :])
                    t1 = rpool.tile([128, CH], F32, tag="t1", name="t1", bufs=1)
                    nc.vector.tensor_tensor(t1[:], ps[:], cc_sb[:, ts], OP.mult)
                    t2 = rpool.tile([128, CH], F32, tag="t2", name="t2", bufs=1)
                    nc.vector.tensor_tensor(t2[:], qsw[:], ss_sb[:, ts], OP.mult)
                    nc.vector.tensor_tensor(dst, t1[:], t2[:], OP.add)

                halves = [
                    [("q0", 0), ("q1", 1), ("q2", 2)],
                    [("q3", 3), ("k", -1), ("v", -2)],
                ]
                for t in range(NCH):
                    ts = slice(t * CH, (t + 1) * CH)
                    if t in prefetched:
                        xt_h = prefetched.pop(t)
                    else:
                        xt_h = load_xt(t)
                    for hi, half in enumerate(halves):
                        pss = {}
                        for slot, (tag, j) in enumerate(half):
                            pst = pspj.tile(
                                [128, CH], F32, tag=f"pj{hi}{slot}",
                                name=f"ps_{tag}",
                            )
                            pss[tag] = pst
                        for kb in range(KB):
                            xt_t = xt_h[kb // (KB // 2)]
                            rhs = xt_t[:, kb % (KB // 2), :]
                            for (tag, j) in half:
                                if j >= 0:
                                    lhsT = wq_sb[:, kb, j * HD:(j + 1) * HD]
                                elif j == -1:
                                    lhsT = wk_sb[:, kb, :]
                                else:
                                    lhsT = wv_sb[:, kb, :]
                                nc.tensor.matmul(
                                    pss[tag][:], lhsT, rhs,
                                    start=(kb == 0), stop=(kb == KB - 1),
                                )
                        for (tag, j) in half:
                            ps = pss[tag]
                            if j >= 0:
                                rope(ps, qT_sb[:, j, ts], ts)
                            elif j == -1:
                                rope(ps, kT_sb[:, ts], ts)
                            else:
                                vtmp = rpool.tile(
                                    [128, CH], F32R, tag="vtmp", name="vtmp"
                                )
                                nc.scalar.copy(vtmp[:], ps[:])
                                for j2 in range(4):
                                    trp = pstr.tile(
                                        [128, 128], F32R, tag="tr", name="trp"
                                    )
                                    nc.tensor.transpose(
                                        trp[:],
                                        vtmp[:, j2 * 128:(j2 + 1) * 128],
                                        eye_sb[:],
                                    )
                                    nc.scalar.copy(
                                        v_sb[:, t * 4 + j2, :], trp[:]
                                    )

            # ---- phases A+O interleaved per query chunk
            with (
                tc.tile_pool(name="wop", bufs=1) as wopool,
                tc.tile_pool(name="attns", bufs=4) as aspool,
                tc.tile_pool(name="probs", bufs=4) as ppool,
                tc.tile_pool(name="small", bufs=2) as spool,
                tc.tile_pool(name="outev", bufs=3) as opool,
                tc.tile_pool(name="psmm", bufs=3, space="PSUM") as psmm,
                tc.tile_pool(name="psat", bufs=3, space="PSUM") as psat,
                tc.tile_pool(name="pssb", bufs=2, space="PSUM") as pssb,
            ):
                wo_sb = wopool.tile([128, HPC, D], F32R)
                for hb in range(HPC):
                    nc.sync.dma_start(wo_sb[:, hb, :], wo_r[:, hb, :])

                pending = []

                def emit_outproj(qc, attn_slice):
                    # out-projection for this chunk's 4 token blocks
                    for mloc in range(4):
                        mg = qc * 4 + mloc
                        for n in range(4):
                            o = psmm.tile([128, CH], F32, tag="mm", name=f"o{n}")
                            for hd in range(HPC):
                                nc.tensor.matmul(
                                    o[:],
                                    attn_slice[:, hd, mloc * 128:(mloc + 1) * 128],
                                    wo_sb[:, hd, n * CH:(n + 1) * CH],
                                    start=(hd == 0),
                                    stop=(hd == HPC - 1),
                                )
                            osb = opool.tile([128, CH], F32, tag="osb", name="osb")
                            if n % 2 == 0:
                                nc.vector.tensor_copy(osb[:], o[:])
                            else:
                                nc.scalar.copy(osb[:], o[:])
                            nc.sync.dma_start(
                                out[mg * 128:(mg + 1) * 128, n * CH:(n + 1) * CH],
                                osb[:],
                            )

                for qc in range(NCH):
                    qs = slice(qc * CH, (qc + 1) * CH)
                    attn_slice = aspool.tile(
                        [128, HPC, CH], F32R, tag="asl", name="attn_slice"
                    )
                    nkb = 4 * qc + 4
                    for pair in range(2):
                        hs = (2 * pair, 2 * pair + 1)
                        attn_ps = {}
                        sums_ps = {}
                        for h in hs:
                            attn_ps[h] = psat.tile(
                                [128, CH], F32, tag="attn", name=f"attn{h}"
                            )
                            sums_ps[h] = pssb.tile(
                                [128, CH], F32, tag="sb", name=f"sums{h}"
                            )
                        for kb in range(nkb):
                            r = kb - 4 * qc
                            lo = 128 * r if r > 0 else 0
                            scs = {}
                            for h in hs:  # shared kT stationary
                                sc = psmm.tile(
                                    [128, CH], F32, tag="mm", name=f"sc{h}"
                                )
                                nc.tensor.matmul(
                                    sc[:, lo:],
                                    kT_sb[:, kb * 128:(kb + 1) * 128],
                                    qT_sb[:, h, qc * CH + lo:(qc + 1) * CH],
                                    start=True,
                                    stop=True,
                                )
                                scs[h] = sc
                            pts = {}
                            for h in hs:
                                pt = ppool.tile(
                                    [128, CH], F32R, tag="pt", name=f"pt{h}"
                                )
                                nc.scalar.activation(
                                    pt[:, lo:], scs[h][:, lo:], AF.Exp
                                )
                                if r >= 0:
                                    nc.vector.tensor_tensor(
                                        pt[:, 128 * r:128 * (r + 1)],
                                        pt[:, 128 * r:128 * (r + 1)],
                                        trim_sb[:],
                                        OP.mult,
                                    )
                                pts[h] = pt
                            for h in hs:  # shared v stationary
                                nc.tensor.matmul(
                                    attn_ps[h][:, lo:], v_sb[:, kb, :],
                                    pts[h][:, lo:],
                                    start=(kb == 0), stop=(kb == nkb - 1),
                                    skip_group_check=True,
                                )
                            for h in hs:  # shared ones stationary
                                nc.tensor.matmul(
                                    sums_ps[h][0:1, lo:], onesc_sb[:],
                                    pts[h][:, lo:],
                                    start=(kb == 0), stop=(kb == nkb - 1),
                                    skip_group_check=True,
                                )
                        for h in hs:
                            # 1/sums = exp(-ln(sums)); broadcast via K=1 matmul
                            r1 = spool.tile([1, CH], F32, tag="r1", name="r1")
                            nc.scalar.activation(
                                r1[:], sums_ps[h][0:1, :], AF.Ln
                            )
                            r2 = spool.tile([1, CH], F32R, tag="r2", name="r2")
                            nc.scalar.activation(r2[:], r1[:], AF.Exp, scale=-1.0)
                            nc.tensor.matmul(
                                sums_ps[h][:], onesr_sb[:], r2[:],
                                start=True, stop=True, skip_group_check=True,
                            )
                            bc_sb = spool.tile(
                                [128, CH], F32, tag="bcs", name="bc_sb"
                            )
                            nc.vector.tensor_copy(bc_sb[:], sums_ps[h][:])
                            nc.vector.tensor_tensor(
                                attn_slice[:, h, :], attn_ps[h][:], bc_sb[:],
                                OP.mult,
                            )
                    pending.append((qc, attn_slice))
                    if len(pending) > 3:
                        emit_outproj(*pending.pop(0))
                while pending:
                    emit_outproj(*pending.pop(0))
    _split_excess_waits(nc)
    return nc


_nc_cache = [None]


def _get_nc():
    if _nc_cache[0] is None:
        _nc_cache[0] = _build()
    return _nc_cache[0]


# ---------------------------------------------------------------- host ----


def kernel(x, freqs_cos, freqs_sin, Wq, Wk, Wv, Wo):
    _install_ntff_shim()
    B = x.shape[0]
    assert x.shape == (B, C, D)
    scale = np.float32(1.0 / np.sqrt(D))
    perm = np.concatenate([np.arange(0, HD, 2), np.arange(1, HD, 2)])

    cosT = np.ascontiguousarray(freqs_cos.T.astype(np.float32))  # (64, C)
    sinT = np.ascontiguousarray(freqs_sin.T.astype(np.float32))
    cc = np.vstack([cosT, cosT])                   # (128, C)
    ss = np.vstack([-sinT, sinT])
    eye = np.eye(128, dtype=np.float32)
    trimask = np.triu(np.ones((128, 128), np.float32))
    onesc = np.ones((128, 1), np.float32)
    onesr = np.ones((1, 128), np.float32)

    in_maps = []
    for core in range(8):
        b, g = core // 4, core % 4
        xT = np.ascontiguousarray(x[b].T.astype(np.float32))
        wq_cols = []
        for hh in range(HPC):
            h = 4 * g + hh
            wq_cols.append(Wq[:, h * HD:(h + 1) * HD][:, perm] * scale)
        wq_c = np.ascontiguousarray(
            np.concatenate(wq_cols, axis=1).astype(np.float32)
        )
        wk_c = np.ascontiguousarray(
            Wk[:, g * HD:(g + 1) * HD][:, perm].astype(np.float32)
        )
        wv_c = np.ascontiguousarray(Wv[:, g * HD:(g + 1) * HD].astype(np.float32))
        wo_c = np.ascontiguousarray(
            Wo[g * HPC * HD:(g + 1) * HPC * HD, :].astype(np.float32)
        )
        in_maps.append(
            {
                "xT": xT,
                "wq": wq_c,
                "wk": wk_c,
                "wv": wv_c,
                "wo": wo_c,
                "cc": cc,
                "ss": ss,
                "eye": eye,
                "trim": trimask,
                "onesc": onesc,
                "onesr": onesr,
            }
        )

    nc = _get_nc()
    res = run_bass_kernel_spmd(nc, in_maps, core_ids=list(range(8)), trace=TRACE)
    kernel.last_results = res

    final = np.zeros((B, C, D), np.float32)
    for core in range(8):
        b = core // 4
        final[b] += res.results[core]["out"]
    return final


# revision 35
# speedup vs baseline: 1.2071x; 1.2071x over previous
"""Causal GQA self-attention (B=2, C=2048, D=2048, 16 heads / 4 KV groups)
as a Bass/Tile SPMD kernel on 8 trn2 NeuronCores.

Sharding: core i handles batch b = i // 4, KV group g = i % 4 (4 query
heads + 1 KV head). Each core computes a partial output
x[b] @ Wq_g -> attention -> attn @ Wo_g_rows; the host sums the 4
partials per batch (the Wo-row split means partials add exactly).

Layout choices (matmuls run as float32r):
- host feeds x transposed (xT: D x C) so both projection operands are
  naturally k-major; projections emit head-dim-major qT/kT/vT tiles.
- Wq/Wk columns are permuted per head to [even dims | odd dims] so RoPE
  becomes full-width elementwise ops: rot = q*CC + swap(q)*SS with
  CC=[cos;cos], SS=[-sin;sin]; swap(q) made by two ACT partition-shift
  copies. The softmax scale 1/sqrt(D) is folded into Wq on the host.
- scores are computed transposed (keys on partitions, queries free) so
  probs feed the attnT matmul without transposes; softmax denominators
  come from a ones-column matmul; 1/sum via ACT exp(-ln(s)); the
  per-query normalizer is broadcast across partitions with a K=1
  outer-product matmul into the same PSUM bank the sums used.
- v alone is transposed (PE transpose via identity) to token-major.
- attention runs head pairs so consecutive matmuls share the stationary
  kT/v/ones operands; causal structure is exploited by computing only
  the valid query subrange of every scores/probs/attn/sums tile.
- DMA order is tuned so the first projection matmul starts ~13us in
  (wq and chunk-0 xT interleaved, then later chunks, then wk/wv; small
  constants ride the idle SWDGE queue).
- out-projection emission is deferred two query chunks behind attention
  so its exp-independent matmuls fill the PE stalls that the
  scores->exp->attn dependency chain would otherwise expose.
"""
import sys
import types

import numpy as np

import concourse.bass as bass
import concourse.mybir as mybir
import concourse.tile as tile
from concourse.bass_utils import run_bass_kernel_spmd

F32R = mybir.dt.float32r
F32 = mybir.dt.float32
AF = mybir.ActivationFunctionType
OP = mybir.AluOpType

C = 2048
D = 2048
HD = 128
HPC = 4          # query heads per core
CH = 512         # token chunk
NCH = C // CH    # 4
KB = D // 128    # 16 k blocks
TRACE = False    # set by test harness to collect NTFF exec time

# ---------------------------------------------------------------- shims ----


def _install_ntff_shim():
    """Provide antenv.axon_hooks (missing on this image) so trace=True works."""
    if "antenv.axon_hooks" in sys.modules:
        return
    try:
        import antenv
        from trn_agent_boot.trn_boot import _ntff_profile_via_ctypes
    except ImportError:
        return
    mod = types.ModuleType("antenv.axon_hooks")
    state = {"hook": None}
    mod.set_axon_ntff_profile_hook = lambda h: state.__setitem__("hook", h)
    mod.get_axon_ntff_profile_hook = lambda: state["hook"]
    sys.modules["antenv.axon_hooks"] = mod
    antenv.axon_hooks = mod
    mod.set_axon_ntff_profile_hook(
        _ntff_profile_via_ctypes("/opt/axon/libaxon_pjrt.so")
    )


_wsplit_counter = [0]


def _split_excess_waits(nc, limit: int = 1):
    """This walrus build allows few sync waits per instruction (fused f32r
    matmuls take one); move extras onto same-engine NoOp carriers."""
    for f in nc.m.functions:
        for bb in f.blocks:
            insts = list(bb.instructions)
            if not any(
                i.sync_info is not None and len(i.sync_info.on_wait) > limit
                for i in insts
            ):
                continue
            new_insts = []
            for inst in insts:
                si = inst.sync_info
                if si is not None and len(si.on_wait) > limit:
                    waits = list(si.on_wait)
                    keep, extra = waits[:limit], waits[limit:]
                    for w in extra:
                        _wsplit_counter[0] += 1
                        nop = mybir.InstNoOp(
                            name=f"I-wsplit-{_wsplit_counter[0]}",
                            ins=[],
                            outs=[],
                            engine=inst.engine,
                        )
                        nop.sync_info = mybir.SyncInfo(on_wait=[w], on_update=[])
                        new_insts.append(nop)
                    inst.sync_info = mybir.SyncInfo(
                        on_wait=keep, on_update=list(si.on_update)
                    )
                new_insts.append(inst)
            bb.instructions = new_insts


# ------------------------------------------------------------- program ----


def _build():
    nc = bass.Bass(target_bir_lowering=False)

    xT = nc.dram_tensor("xT", [D, C], F32R, kind="ExternalInput")
    wq = nc.dram_tensor("wq", [D, HPC * HD], F32R, kind="ExternalInput")
    wk = nc.dram_tensor("wk", [D, HD], F32R, kind="ExternalInput")
    wv = nc.dram_tensor("wv", [D, HD], F32R, kind="ExternalInput")
    wo = nc.dram_tensor("wo", [HPC * HD, D], F32R, kind="ExternalInput")
    cc = nc.dram_tensor("cc", [128, C], F32, kind="ExternalInput")
    ss = nc.dram_tensor("ss", [128, C], F32, kind="ExternalInput")
    eye = nc.dram_tensor("eye", [128, 128], F32R, kind="ExternalInput")
    trim = nc.dram_tensor("trim", [128, 128], F32R, kind="ExternalInput")
    onesc = nc.dram_tensor("onesc", [128, 1], F32R, kind="ExternalInput")
    onesr = nc.dram_tensor("onesr", [1, 128], F32R, kind="ExternalInput")
    out = nc.dram_tensor("out", [C, D], F32, kind="ExternalOutput")

    wq_r = wq.rearrange("(kb p) n -> p kb n", p=128)
    wk_r = wk.rearrange("(kb p) n -> p kb n", p=128)
    wv_r = wv.rearrange("(kb p) n -> p kb n", p=128)
    wo_r = wo.rearrange("(hb p) e -> p hb e", p=128)
    xT_r = xT.rearrange("(kb p) m -> p kb m", p=128)

    with tile.TileContext(nc) as tc:
        with (
            tc.tile_pool(name="const", bufs=1) as cpool,
            tc.tile_pool(name="acts", bufs=1) as apool,
        ):
            # ---- resident constants / activations (data DMAs emitted in
            # phase P interleaved with xT so the first matmul starts early;
            # low-priority constants go on the gpsimd SWDGE queue)
            wk_sb = cpool.tile([128, KB, HD], F32R)
            wv_sb = cpool.tile([128, KB, HD], F32R)
            cc_sb = cpool.tile([128, C], F32)
            ss_sb = cpool.tile([128, C], F32)
            eye_sb = cpool.tile([128, 128], F32R)
            nc.gpsimd.dma_start(eye_sb[:], eye[:])
            trim_sb = cpool.tile([128, 128], F32R)
            nc.gpsimd.dma_start(trim_sb[:], trim[:])
            onesc_sb = cpool.tile([128, 1], F32R)
            nc.gpsimd.dma_start(onesc_sb[:], onesc[:])
            onesr_sb = cpool.tile([1, 128], F32R)
            nc.gpsimd.dma_start(onesr_sb[:], onesr[:])

            qT_sb = apool.tile([128, HPC, C], F32R)    # head-dim major q
            kT_sb = apool.tile([128, C], F32R)         # head-dim major k
            v_sb = apool.tile([128, KB, HD], F32R)     # token-major v

            # ---- phase P: projections + RoPE, two chunks per weight load
            with (
                tc.tile_pool(name="wqp", bufs=1) as wqpool,
                tc.tile_pool(name="xt", bufs=4) as xtpool,
                tc.tile_pool(name="rope", bufs=2) as rpool,
                tc.tile_pool(name="psproj", bufs=1, space="PSUM") as pspj,
                tc.tile_pool(name="pstr", bufs=1, space="PSUM") as pstr,
            ):
                wq_sb = wqpool.tile([128, KB, HPC * HD], F32R)

                def load_xt(t):
                    halves_x = []
                    for xh in range(2):
                        xt_t = xtpool.tile(
                            [128, KB // 2, CH], F32R, tag="xt", name="xt"
                        )
                        for kb2 in range(KB // 2):
                            kb = xh * (KB // 2) + kb2
                            nc.sync.dma_start(
                                xt_t[:, kb2, :],
                                xT_r[:, kb, t * CH:(t + 1) * CH],
                            )
                        halves_x.append(xt_t)
                    return halves_x

                # DMA order tuned for an early first matmul: wq and chunk-0
                # xT interleaved, then chunk-1 xT, then wk/wv (used only by
                # the second half-pass), cc/ss on the idle SWDGE queue.
                xt0a = xtpool.tile([128, KB // 2, CH], F32R, tag="xt", name="xt")
                xt0b = xtpool.tile([128, KB // 2, CH], F32R, tag="xt", name="xt")
                xt0 = [xt0a, xt0b]
                for kb in range(KB):
                    nc.sync.dma_start(wq_sb[:, kb, :], wq_r[:, kb, :])
                    nc.sync.dma_start(
                        xt0[kb // (KB // 2)][:, kb % (KB // 2), :],
                        xT_r[:, kb, 0:CH],
                    )
                nc.gpsimd.dma_start(cc_sb[:], cc[:])
                nc.gpsimd.dma_start(ss_sb[:], ss[:])
                prefetched = {0: xt0, 1: load_xt(1)}
                for kb in range(KB):
                    nc.sync.dma_start(wk_sb[:, kb, :], wk_r[:, kb, :])
                    nc.sync.dma_start(wv_sb[:, kb, :], wv_r[:, kb, :])

                def rope(ps, dst, ts):
                    qsw = rpool.tile([128, CH], F32, tag="qsw", name="qsw")
                    nc.scalar.copy(qsw[0:64, :], ps[64:128, :])
                    nc.scalar.copy(qsw[64:128, :], ps[0:64, :])
                    t1 = rpool.tile([128, CH], F32, tag="t1", name="t1", bufs=1)
                    nc.vector.tensor_tensor(t1[:], ps[:], cc_sb[:, ts], OP.mult)
                    t2 = rpool.tile([128, CH], F32, tag="t2", name="t2", bufs=1)
                    nc.vector.tensor_tensor(t2[:], qsw[:], ss_sb[:, ts], OP.mult)
                    nc.vector.tensor_tensor(dst, t1[:], t2[:], OP.add)

                halves = [
                    [("q0", 0), ("q1", 1), ("q2", 2)],
                    [("q3", 3), ("k", -1), ("v", -2)],
                ]
                for t in range(NCH):
                    ts = slice(t * CH, (t + 1) * CH)
                    if t in prefetched:
                        xt_h = prefetched.pop(t)
                    else:
                        xt_h = load_xt(t)
                    for hi, half in enumerate(halves):
                        pss = {}
                        for slot, (tag, j) in enumerate(half):
                            pst = pspj.tile(
                                [128, CH], F32, tag=f"pj{hi}{slot}",
                                name=f"ps_{tag}",
                            )
                            pss[tag] = pst
                        for kb in range(KB):
                            xt_t = xt_h[kb // (KB // 2)]
                            rhs = xt_t[:, kb % (KB // 2), :]
                            for (tag, j) in half:
                                if j >= 0:
                                    lhsT = wq_sb[:, kb, j * HD:(j + 1) * HD]
                                elif j == -1:
                                    lhsT = wk_sb[:, kb, :]
                                else:
                                    lhsT = wv_sb[:, kb, :]
                                nc.tensor.matmul(
                                    pss[tag][:], lhsT, rhs,
                                    start=(kb == 0), stop=(kb == KB - 1),
                                )
                        for (tag, j) in half:
                            ps = pss[tag]
                            if j >= 0:
                                rope(ps, qT_sb[:, j, ts], ts)
                            elif j == -1:
                                rope(ps, kT_sb[:, ts], ts)
                            else:
                                vtmp = rpool.tile(
                                    [128, CH], F32R, tag="vtmp", name="vtmp"
                                )
                                nc.scalar.copy(vtmp[:], ps[:])
                                for j2 in range(4):
                                    trp = pstr.tile(
                                        [128, 128], F32R, tag="tr", name="trp"
                                    )
                                    nc.tensor.transpose(
                                        trp[:],
                                        vtmp[:, j2 * 128:(j2 + 1) * 128],
                                        eye_sb[:],
                                    )
                                    nc.scalar.copy(
                                        v_sb[:, t * 4 + j2, :], trp[:]
                                    )

            # ---- phases A+O interleaved per query chunk
            with (
                tc.tile_pool(name="wop", bufs=1) as wopool,
                tc.tile_pool(name="attns", bufs=3) as aspool,
                tc.tile_pool(name="probs", bufs=4) as ppool,
                tc.tile_pool(name="small", bufs=2) as spool,
                tc.tile_pool(name="outev", bufs=3) as opool,
                tc.tile_pool(name="psmm", bufs=3, space="PSUM") as psmm,
                tc.tile_pool(name="psat", bufs=3, space="PSUM") as psat,
                tc.tile_pool(name="pssb", bufs=2, space="PSUM") as pssb,
            ):
                wo_sb = wopool.tile([128, HPC, D], F32R)
                for hb in range(HPC):
                    nc.sync.dma_start(wo_sb[:, hb, :], wo_r[:, hb, :])

                pending = []

                def emit_outproj(qc, attn_slice):
                    # out-projection for this chunk's 4 token blocks
                    for mloc in range(4):
                        mg = qc * 4 + mloc
                        for n in range(4):
                            o = psmm.tile([128, CH], F32, tag="mm", name=f"o{n}")
                            for hd in range(HPC):
                                nc.tensor.matmul(
                                    o[:],
                                    attn_slice[:, hd, mloc * 128:(mloc + 1) * 128],
                                    wo_sb[:, hd, n * CH:(n + 1) * CH],
                                    start=(hd == 0),
                                    stop=(hd == HPC - 1),
                                )
                            osb = opool.tile([128, CH], F32, tag="osb", name="osb")
                            if n % 2 == 0:
                                nc.vector.tensor_copy(osb[:], o[:])
                            else:
                                nc.scalar.copy(osb[:], o[:])
                            nc.sync.dma_start(
                                out[mg * 128:(mg + 1) * 128, n * CH:(n + 1) * CH],
                                osb[:],
                            )

                for qc in range(NCH):
                    qs = slice(qc * CH, (qc + 1) * CH)
                    attn_slice = aspool.tile(
                        [128, HPC, CH], F32R, tag="asl", name="attn_slice"
                    )
                    nkb = 4 * qc + 4
                    for pair in range(2):
                        hs = (2 * pair, 2 * pair + 1)
                        attn_ps = {}
                        sums_ps = {}
                        for h in hs:
                            attn_ps[h] = psat.tile(
                                [128, CH], F32, tag="attn", name=f"attn{h}"
                            )
                            sums_ps[h] = pssb.tile(
                                [128, CH], F32, tag="sb", name=f"sums{h}"
                            )
                        for kb in range(nkb):
                            r = kb - 4 * qc
                            lo = 128 * r if r > 0 else 0
                            scs = {}
                            for h in hs:  # shared kT stationary
                                sc = psmm.tile(
                                    [128, CH], F32, tag="mm", name=f"sc{h}"
                                )
                                nc.tensor.matmul(
                                    sc[:, lo:],
                                    kT_sb[:, kb * 128:(kb + 1) * 128],
                                    qT_sb[:, h, qc * CH + lo:(qc + 1) * CH],
                                    start=True,
                                    stop=True,
                                )
                                scs[h] = sc
                            pts = {}
                            for h in hs:
                                pt = ppool.tile(
                                    [128, CH], F32R, tag="pt", name=f"pt{h}"
                                )
                                nc.scalar.activation(
                                    pt[:, lo:], scs[h][:, lo:], AF.Exp
                                )
                                if r >= 0:
                                    nc.vector.tensor_tensor(
                                        pt[:, 128 * r:128 * (r + 1)],
                                        pt[:, 128 * r:128 * (r + 1)],
                                        trim_sb[:],
                                        OP.mult,
                                    )
                                pts[h] = pt
                            for h in hs:  # shared v stationary
                                nc.tensor.matmul(
                                    attn_ps[h][:, lo:], v_sb[:, kb, :],
                                    pts[h][:, lo:],
                                    start=(kb == 0), stop=(kb == nkb - 1),
                                    skip_group_check=True,
                                )
                            for h in hs:  # shared ones stationary
                                nc.tensor.matmul(
                                    sums_ps[h][0:1, lo:], onesc_sb[:],
                                    pts[h][:, lo:],
                                    start=(kb == 0), stop=(kb == nkb - 1),
                                    skip_group_check=True,
                                )
                        for h in hs:
                            # 1/sums = exp(-ln(sums)); broadcast via K=1 matmul
                            r1 = spool.tile([1, CH], F32, tag="r1", name="r1")
                            nc.scalar.activation(
                                r1[:], sums_ps[h][0:1, :], AF.Ln
                            )
                            r2 = spool.tile([1, CH], F32R, tag="r2", name="r2")
                            nc.scalar.activation(r2[:], r1[:], AF.Exp, scale=-1.0)
                            nc.tensor.matmul(
                                sums_ps[h][:], onesr_sb[:], r2[:],
                                start=True, stop=True, skip_group_check=True,
                            )
                            bc_sb = spool.tile(
                                [128, CH], F32, tag="bcs", name="bc_sb"
                            )
                            nc.vector.tensor_copy(bc_sb[:], sums_ps[h][:])
                            nc.vector.tensor_tensor(
                                attn_slice[:, h, :], attn_ps[h][:], bc_sb[:],
                                OP.mult,
                            )
                    pending.append((qc, attn_slice))
                    if len(pending) > 2:
                        emit_outproj(*pending.pop(0))
                while pending:
                    emit_outproj(*pending.pop(0))
    _split_excess_waits(nc)
    return nc


_nc_cache = [None]


def _get_nc():
    if _nc_cache[0] is None:
        _nc_cache[0] = _build()
    return _nc_cache[0]


# ---------------------------------------------------------------- host ----


def kernel(x, freqs_cos, freqs_sin, Wq, Wk, Wv, Wo):
    _install_ntff_shim()
    B = x.shape[0]
    assert x.shape == (B, C, D)
    scale = np.float32(1.0 / np.sqrt(D))
    perm = np.concatenate([np.arange(0, HD, 2), np.arange(1, HD, 2)])

    cosT = np.ascontiguousarray(freqs_cos.T.astype(np.float32))  # (64, C)
    sinT = np.ascontiguousarray(freqs_sin.T.astype(np.float32))
    cc = np.vstack([cosT, cosT])                   # (128, C)
    ss = np.vstack([-sinT, sinT])
    eye = np.eye(128, dtype=np.float32)
    trimask = np.triu(np.ones((128, 128), np.float32))
    onesc = np.ones((128, 1), np.float32)
    onesr = np.ones((1, 128), np.float32)

    in_maps = []
    for core in range(8):
        b, g = core // 4, core % 4
        xT = np.ascontiguousarray(x[b].T.astype(np.float32))
        wq_cols = []
        for hh in range(HPC):
            h = 4 * g + hh
            wq_cols.append(Wq[:, h * HD:(h + 1) * HD][:, perm] * scale)
        wq_c = np.ascontiguousarray(
            np.concatenate(wq_cols, axis=1).astype(np.float32)
        )
        wk_c = np.ascontiguousarray(
            Wk[:, g * HD:(g + 1) * HD][:, perm].astype(np.float32)
        )
        wv_c = np.ascontiguousarray(Wv[:, g * HD:(g + 1) * HD].astype(np.float32))
        wo_c = np.ascontiguousarray(
            Wo[g * HPC * HD:(g + 1) * HPC * HD, :].astype(np.float32)
        )
        in_maps.append(
            {
                "xT": xT,
                "wq": wq_c,
                "wk": wk_c,
                "wv": wv_c,
                "wo": wo_c,
                "cc": cc,
                "ss": ss,
                "eye": eye,
                "trim": trimask,
                "onesc": onesc,
                "onesr": onesr,
            }
        )

    nc = _get_nc()
    res = run_bass_kernel_spmd(nc, in_maps, core_ids=list(range(8)), trace=TRACE)
    kernel.last_results = res

    final = np.zeros((B, C, D), np.float32)
    for core in range(8):
        b = core // 4
        final[b] += res.results[core]["out"]
    return final
